# revision 1
# baseline (speedup 1.0000x reference)
"""Trainium2 Bass kernel for nn_ClassifierModel_87883620811309 (detection loss).

Strategy (data-parallel over images, 8 cores x 4 images):
  Per image the dominant work is a [128 labels x 16384 proposals] IoU-argmax.
  Per (label l, proposal n):   iou = inter / (areaA + areaB - inter)
  argmax_n iou == argmax_n inter/(areaA+areaB)  (monotone transform), and we
  compare in log domain:  score = ln(inter + 1e-35) - ln(areaA + areaB).

  inter is built from relu-differences:
     iw = relu(wA - (relu(ax2-bx2) + relu(bx1-ax1)))   (same for y)
     inter = iw*ih  (computed as tx*ty with tx=-iw, ty=-ih)

  Proposal-side rows (bx1,bx2,by1,by2,areaB) are broadcast across the 128
  label partitions by the TensorEngine: K=3 matmul of an all-ones [3,128]
  bf16 lhsT against 3-way bf16-split rows (exact fp32 reconstruction in PSUM).
  The ScalarEngine consumes PSUM with fused scale/bias/relu(/ln).  The row
  max + argmax come from a fused tensor_tensor_reduce + max_index (first-tie
  semantics match jnp.argmax).

  Everything else (scatter-min dedup of labels onto proposals, huber on the
  <=128 matched proposals, sigmoid-sum for the CCE term, L2 sums) is tiny and
  done per image with [128,1]-level ops, indirect DMA gathers, and a PE
  partition-sum.  Each core emits one scalar partial loss; the host adds the
  8 partials plus the closed-form constant 32*N*(-ln(eps)).
"""

import os
import sys

for p in ("/opt/trn_rl_repo", "/opt/pypackages"):
    if os.path.isdir(p) and p not in sys.path:
        sys.path.insert(0, p)

import numpy as np

import concourse.bass as bass
import concourse.bacc as bacc
import concourse.tile as tile
from concourse import mybir
from concourse.bass_utils import run_bass_kernel_spmd

dt = mybir.dt
Alu = mybir.AluOpType
Act = mybir.ActivationFunctionType

N_CORES = 8
BATCH = 32
IMGS = BATCH // N_CORES          # 4 images per core
N = 16384                        # proposals
L = 128                          # labels
STRIDE = 16.0
LOG_EPS = 1e-10
CCE_EPS = 1e-7
LOG_LO = float(np.log(CCE_EPS))          # ~ -16.118
LOG_HI = float(np.log1p(-CCE_EPS))       # ~ -1e-7
DLH = LOG_LO - LOG_HI                    # lo - hh
CHUNK = 512
NCHUNK = N // CHUNK              # 32

_CACHED = {}


def _build_nc():
    nc = bacc.Bacc("TRN2", target_bir_lowering=False, debug=False,
                   num_devices=N_CORES)

    b5_d = nc.dram_tensor("b5", [IMGS, 5, N], dt.float32,
                          kind="ExternalInput")
    lab_d = nc.dram_tensor("labels", [IMGS, L, 4], dt.float32,
                           kind="ExternalInput")
    t_d = nc.dram_tensor("gtab", [IMGS * N, 10], dt.float32,
                         kind="ExternalInput")
    cls_d = nc.dram_tensor("cls", [IMGS, 2, 128, 128], dt.float32,
                           kind="ExternalInput")
    bbox_d = nc.dram_tensor("bbox", [IMGS, 128, 512], dt.float32,
                            kind="ExternalInput")
    ident_d = nc.dram_tensor("ident", [128, 128], dt.float32,
                             kind="ExternalInput")
    ltm_d = nc.dram_tensor("ltm", [128, 128], dt.float32,
                           kind="ExternalInput")
    loss_d = nc.dram_tensor("loss", [1, 1], dt.float32, kind="ExternalOutput")
    dbgm_d = nc.dram_tensor("dbg_match", [IMGS, 128], dt.float32,
                            kind="ExternalOutput")

    K1 = 0.5 / (10.0 * 2 * N)     # cls l2 scale (per image)
    K2 = 0.5 / (4 * N)            # bbox l2 scale

    with tile.TileContext(nc) as tc:
        with tc.tile_pool(name="sb", bufs=2) as sb, \
             tc.tile_pool(name="sbbig", bufs=1) as sbbig, \
             tc.tile_pool(name="sbsm", bufs=2) as sbsm, \
             tc.tile_pool(name="psmisc", bufs=1, space="PSUM") as psmisc:

            ident = sbbig.tile([128, 128], dt.float32)
            nc.sync.dma_start(ident[:], ident_d[:])
            ltm = sbbig.tile([128, 128], dt.float32)
            nc.sync.dma_start(ltm[:], ltm_d[:])
            eps35 = sbbig.tile([128, 1], dt.float32)
            nc.vector.memset(eps35[:], 1e-35)
            onescol = sbbig.tile([128, 1], dt.float32)
            nc.vector.memset(onescol[:], 1.0)
            acc = sbbig.tile([128, 1], dt.float32)
            nc.vector.memset(acc[:], 0.0)

            _reps = int(os.environ.get("BASSK_REPS", "1"))
            for i in list(range(IMGS)) * _reps:
                # ---------------- pairwise phase ----------------
                lab = sb.tile([L, 4], dt.float32, tag="lab")
                nc.sync.dma_start(lab[:], lab_d[i])

                ax1 = lab[:, 0:1]
                ay1 = lab[:, 1:2]
                wA = lab[:, 2:3]
                hA = lab[:, 3:4]
                scal = sb.tile([L, 8], dt.float32, tag="scal")
                nc.vector.tensor_tensor(scal[:, 0:1], ax1, wA, Alu.add)    # ax2
                nc.vector.tensor_tensor(scal[:, 1:2], ay1, hA, Alu.add)    # ay2
                nc.vector.tensor_tensor(scal[:, 4:5], wA, hA, Alu.mult)    # areaA

                score = sbbig.tile([128, N], dt.float32, tag="score")
                segmax = sb.tile([128, NCHUNK], dt.float32, tag="segmax")

                CH = 2048
                _nopair = os.environ.get("BASSK_NOPAIR") == "1"
                for c in ([] if _nopair else range(N // CH)):
                    sl = slice(CH * c, CH * (c + 1))
                    bc = sb.tile([128, 5, CH], dt.float32, tag="bc", bufs=1)
                    nc.sync.dma_start(bc[:],
                                      b5_d[i:i + 1, :, sl].to_broadcast([128, 5, CH]))
                    t1 = sb.tile([128, CH], dt.float32, tag="t1")
                    nc.vector.tensor_scalar(t1[:], bc[:, 1, :], scal[:, 0:1],
                                            None, Alu.min)          # min(bx2, ax2)
                    t2 = sb.tile([128, CH], dt.float32, tag="t2")
                    nc.vector.tensor_scalar(t2[:], bc[:, 0, :], ax1,
                                            None, Alu.max)          # max(bx1, ax1)
                    nc.vector.tensor_tensor(t1[:], t1[:], t2[:], Alu.subtract)
                    nc.vector.tensor_scalar(t1[:], t1[:], 0.0, None, Alu.max)
                    t3 = sb.tile([128, CH], dt.float32, tag="t3")
                    nc.vector.tensor_scalar(t3[:], bc[:, 3, :], scal[:, 1:2],
                                            None, Alu.min)          # min(by2, ay2)
                    nc.vector.tensor_scalar(t2[:], bc[:, 2, :], ay1,
                                            None, Alu.max)          # max(by1, ay1)
                    nc.vector.tensor_tensor(t3[:], t3[:], t2[:], Alu.subtract)
                    nc.vector.tensor_scalar(t3[:], t3[:], 0.0, None, Alu.max)
                    nc.vector.tensor_tensor(t1[:], t1[:], t3[:], Alu.mult)  # inter
                    li = sb.tile([128, CH], dt.float32, tag="li")
                    nc.scalar.activation(li[:], t1[:], Act.Ln,
                                         bias=eps35[:, 0:1], scale=1.0)
                    ls = sb.tile([128, CH], dt.float32, tag="ls")
                    nc.scalar.activation(ls[:], bc[:, 4, :], Act.Ln,
                                         bias=scal[:, 4:5], scale=1.0)
                    nc.vector.tensor_tensor(score[:, sl], li[:], ls[:],
                                            Alu.subtract)
                if _nopair:
                    nc.vector.memset(score[:], 0.0)
                nc.vector.tensor_reduce(
                    segmax[:], score[:].rearrange("p (c f) -> p c f", c=NCHUNK),
                    mybir.AxisListType.X, Alu.max)
                rmax = sb.tile([128, 1], dt.float32, tag="rmax")
                nc.vector.tensor_reduce(rmax[:], segmax[:], mybir.AxisListType.X,
                                        Alu.max)
                in8 = sb.tile([128, 8], dt.float32, tag="in8")
                nc.vector.tensor_copy(in8[:], rmax[:, 0:1].to_broadcast([128, 8]))
                idx8 = sb.tile([128, 8], dt.uint32, tag="idx8")
                nc.vector.max_index(idx8[:], in8[:], score[:])
                matchf = sb.tile([128, 1], dt.float32, tag="matchf")
                nc.vector.tensor_copy(matchf[:], idx8[:, 0:1])
                nc.sync.dma_start(dbgm_d[i:i+1, :].rearrange("one f -> f one"), matchf[:])

                if os.environ.get("BASSK_NOSMALL") == "1":
                    continue
                # ---------------- small phase ----------------
                sabs = sb.tile([128, 1], dt.float32, tag="sabs")
                nc.vector.tensor_reduce(sabs[:], lab[:], mybir.AxisListType.X,
                                        Alu.add, apply_absolute_value=True)
                validf = sb.tile([128, 1], dt.float32, tag="validf")
                nc.vector.tensor_scalar(validf[:], sabs[:], 0.0, None, Alu.is_gt)
                inv16k = sb.tile([128, 1], dt.float32, tag="inv16k")
                nc.vector.tensor_scalar(inv16k[:], validf[:], -float(N), float(N),
                                        Alu.mult, Alu.add)
                candf = sb.tile([128, 1], dt.float32, tag="candf")
                nc.vector.tensor_scalar(candf[:], matchf[:], validf[:, 0:1],
                                        inv16k[:, 0:1], Alu.mult, Alu.add)
                gidxf = sb.tile([128, 1], dt.float32, tag="gidxf")
                nc.vector.tensor_scalar(gidxf[:], candf[:], float(N - 1),
                                        float(i * N), Alu.min, Alu.add)
                gidx = sb.tile([128, 1], dt.uint32, tag="gidx")
                nc.vector.tensor_copy(gidx[:], gidxf[:])

                gt = sb.tile([128, 10], dt.float32, tag="gt")
                if os.environ.get("BASSK_NOGATHER") == "1":
                    nc.vector.memset(gt[:], 1.0)
                else:
                    nc.gpsimd.indirect_dma_start(
                        out=gt[:], out_offset=None, in_=t_d[:],
                        in_offset=bass.IndirectOffsetOnAxis(ap=gidx[:, 0:1], axis=0))
                roig = gt[:, 0:4]    # rx, ry, rw, rh (image coords)
                bbg = gt[:, 4:8]     # bbox[k::N][n]
                clg = gt[:, 8:10]    # c0[n], c1[n]

                # first-occurrence dedup: label is rep iff valid and no valid
                # earlier label matched the same proposal.  cand of invalid
                # labels is N which never equals a valid cand.
                candT = psmisc.tile([128, 128], dt.float32, tag="m128")
                nc.tensor.transpose(out=candT[:],
                                    in_=candf[:, 0:1].to_broadcast([128, 128]),
                                    identity=ident[:])
                eqm = sb.tile([128, 128], dt.float32, tag="eqm")
                nc.vector.tensor_tensor(eqm[:],
                                        candf[:, 0:1].to_broadcast([128, 128]),
                                        candT[:], Alu.is_equal)
                junk = sb.tile([128, 128], dt.float32, tag="junk")
                notfirst = sb.tile([128, 1], dt.float32, tag="notfirst")
                nc.vector.tensor_tensor(junk[:], eqm[:], ltm[:], Alu.mult)
                nc.vector.tensor_reduce(notfirst[:], junk[:],
                                        mybir.AxisListType.X, Alu.max)
                repf = sb.tile([128, 1], dt.float32, tag="repf")
                nc.vector.tensor_scalar(repf[:], notfirst[:], -1.0, 1.0,
                                        Alu.mult, Alu.add)
                nc.vector.tensor_tensor(repf[:], repf[:], validf[:], Alu.mult)

                # huber targets
                tgt = sb.tile([128, 4], dt.float32, tag="tgt")
                tmp4 = sb.tile([128, 4], dt.float32, tag="tmp4")
                # t0 = (lx - rx)/rw ; t1 = (ly - ry)/rh
                nc.vector.tensor_tensor(tmp4[:, 0:1], lab[:, 0:1], roig[:, 0:1],
                                        Alu.subtract)
                nc.vector.tensor_tensor(tmp4[:, 1:2], lab[:, 1:2], roig[:, 1:2],
                                        Alu.subtract)
                rcp = sb.tile([128, 2], dt.float32, tag="rcp")
                nc.vector.reciprocal(rcp[:], roig[:, 2:4])
                nc.vector.tensor_tensor(tgt[:, 0:1], tmp4[:, 0:1], rcp[:, 0:1],
                                        Alu.mult)
                nc.vector.tensor_tensor(tgt[:, 1:2], tmp4[:, 1:2], rcp[:, 1:2],
                                        Alu.mult)
                # t2 = ln(max(lw/rw, eps)) ; t3 = ln(max(lh/rh, eps))
                nc.vector.tensor_tensor(tmp4[:, 2:3], lab[:, 2:3], rcp[:, 0:1],
                                        Alu.mult)
                nc.vector.tensor_tensor(tmp4[:, 3:4], lab[:, 3:4], rcp[:, 1:2],
                                        Alu.mult)
                rat = sb.tile([128, 2], dt.float32, tag="rat")
                nc.vector.tensor_scalar(rat[:], tmp4[:, 2:4], LOG_EPS, None,
                                        Alu.max)
                nc.scalar.activation(tgt[:, 2:4], rat[:], Act.Ln,
                                     bias=0.0, scale=1.0)

                err = sb.tile([128, 4], dt.float32, tag="err")
                nc.vector.tensor_tensor(err[:], tgt[:], bbg[:], Alu.subtract)
                aerr = sb.tile([128, 4], dt.float32, tag="aerr")
                nc.scalar.activation(aerr[:], err[:], Act.Abs, bias=0.0,
                                     scale=1.0)
                q2 = sb.tile([128, 4], dt.float32, tag="q2")
                nc.vector.tensor_tensor(q2[:], err[:], err[:], Alu.mult)
                nc.vector.tensor_scalar(q2[:], q2[:], 0.5, None, Alu.mult)
                lin = sb.tile([128, 4], dt.float32, tag="lin")
                nc.vector.tensor_scalar(lin[:], aerr[:], -0.5, None, Alu.add)
                small = sb.tile([128, 4], dt.uint8, tag="small")
                nc.vector.tensor_scalar(small[:], aerr[:], 1.0, None, Alu.is_le)
                hcomp = sb.tile([128, 4], dt.float32, tag="hcomp")
                nc.vector.select(hcomp[:], small[:], q2[:], lin[:])
                hub = sb.tile([128, 1], dt.float32, tag="hub")
                nc.vector.tensor_reduce(hub[:], hcomp[:], mybir.AxisListType.X,
                                        Alu.add)
                nc.vector.tensor_scalar(hub[:], hub[:], 0.25, None, Alu.mult)

                # cce correction at matched proposals: DLH*(1-2*p0)
                zg = sb.tile([128, 1], dt.float32, tag="zg")
                nc.vector.tensor_tensor(zg[:], clg[:, 0:1], clg[:, 1:2],
                                        Alu.subtract)
                p0g = sb.tile([128, 1], dt.float32, tag="p0g")
                nc.scalar.activation(p0g[:], zg[:], Act.Sigmoid, bias=0.0,
                                     scale=1.0)
                dl = sb.tile([128, 1], dt.float32, tag="dl")
                nc.vector.tensor_scalar(dl[:], p0g[:], -2.0 * DLH, DLH,
                                        Alu.mult, Alu.add)

                contrib = sb.tile([128, 1], dt.float32, tag="contrib")
                nc.vector.tensor_tensor(contrib[:], hub[:], dl[:], Alu.add)
                nc.vector.tensor_tensor(contrib[:], contrib[:], repf[:], Alu.mult)
                nc.vector.tensor_tensor(acc[:], acc[:], contrib[:], Alu.add)

                # ---------------- cce-full + l2 ----------------
                cpt = sb.tile([128, 2, 128], dt.float32, tag="cpt")
                nc.sync.dma_start(cpt[:], cls_d[i].rearrange("two p f -> p two f"))
                z128 = sb.tile([128, 128], dt.float32, tag="z128")
                nc.vector.tensor_tensor(z128[:], cpt[:, 0, :], cpt[:, 1, :],
                                        Alu.subtract)
                zs = sb.tile([128, 128], dt.float32, tag="zs")
                sp0 = sb.tile([128, 1], dt.float32, tag="sp0")
                nc.scalar.activation(zs[:], z128[:], Act.Sigmoid, bias=0.0,
                                     scale=1.0, accum_out=sp0[:])
                nc.vector.tensor_scalar(sp0[:], sp0[:], DLH, None, Alu.mult)
                nc.vector.tensor_tensor(acc[:], acc[:], sp0[:], Alu.add)

                cflat = cpt[:].rearrange("p two f -> p (two f)")
                jc = sb.tile([128, 256], dt.float32, tag="jc")
                l2c = sb.tile([128, 1], dt.float32, tag="l2c")
                nc.scalar.activation(jc[:], cflat, Act.Square, bias=0.0,
                                     scale=1.0, accum_out=l2c[:])
                nc.vector.tensor_scalar(l2c[:], l2c[:], K1, None, Alu.mult)
                nc.vector.tensor_tensor(acc[:], acc[:], l2c[:], Alu.add)

                bbt = sb.tile([128, 512], dt.float32, tag="bbt")
                nc.sync.dma_start(bbt[:], bbox_d[i])
                jb = sb.tile([128, 512], dt.float32, tag="jb")
                l2b = sb.tile([128, 1], dt.float32, tag="l2b")
                nc.scalar.activation(jb[:], bbt[:], Act.Square, bias=0.0,
                                     scale=1.0, accum_out=l2b[:])
                nc.vector.tensor_scalar(l2b[:], l2b[:], K2, None, Alu.mult)
                nc.vector.tensor_tensor(acc[:], acc[:], l2b[:], Alu.add)

            # partition-sum of acc via PE: ones[128,1].T @ acc -> [1,1]
            tot = psmisc.tile([1, 1], dt.float32, tag="tot")
            nc.tensor.matmul(tot[:], onescol[:, 0:1], acc[:, 0:1],
                             start=True, stop=True)
            lossT = sbbig.tile([1, 1], dt.float32)
            nc.vector.tensor_copy(lossT[:], tot[:])
            nc.sync.dma_start(loss_d[:], lossT[:])

    nc.compile()
    return nc


def _prep_core_inputs(cls, bbox, roi, labels, core):
    sl = slice(core * IMGS, (core + 1) * IMGS)
    cls_c = np.ascontiguousarray(cls[sl]).astype(np.float32)      # [IMGS, 32768]
    bbox_c = np.ascontiguousarray(bbox[sl]).astype(np.float32)    # [IMGS, 65536]
    roi_c = np.ascontiguousarray(roi[sl]).astype(np.float32)      # [IMGS, N, 4]
    lab_c = np.ascontiguousarray(labels[sl]).astype(np.float32)   # [IMGS, L, 4]

    rimg = roi_c * STRIDE
    b5 = np.stack([rimg[..., 0], rimg[..., 0] + rimg[..., 2],
                   rimg[..., 1], rimg[..., 1] + rimg[..., 3],
                   rimg[..., 2] * rimg[..., 3]], axis=1).astype(np.float32)

    # gather table: [IMGS*N, 10] = roi_img(4) | bboxT(4) | clsP(2)
    tgt = np.empty((IMGS, N, 10), dtype=np.float32)
    tgt[..., 0:4] = roi_c * STRIDE
    tgt[..., 4:8] = bbox_c.reshape(IMGS, 4, N).transpose(0, 2, 1)
    tgt[..., 8:10] = cls_c.reshape(IMGS, 2, N).transpose(0, 2, 1)

    ident = np.eye(128, dtype=np.float32)
    ltm = (np.arange(128)[None, :] < np.arange(128)[:, None]).astype(np.float32)

    return {
        "b5": np.ascontiguousarray(b5),
        "labels": lab_c,
        "gtab": np.ascontiguousarray(tgt.reshape(IMGS * N, 10)),
        "cls": np.ascontiguousarray(cls_c.reshape(IMGS, 2, 128, 128)),
        "bbox": np.ascontiguousarray(bbox_c.reshape(IMGS, 128, 512)),
        "ident": ident,
        "ltm": ltm,
    }


def kernel(cls, bbox, roi, labels, _trace=False):
    cls = np.asarray(cls, dtype=np.float32)
    bbox = np.asarray(bbox, dtype=np.float32)
    roi = np.asarray(roi, dtype=np.float32)
    labels = np.asarray(labels, dtype=np.float32)

    if "nc" not in _CACHED:
        _CACHED["nc"] = _build_nc()
    nc = _CACHED["nc"]

    in_maps = [_prep_core_inputs(cls, bbox, roi, labels, k)
               for k in range(N_CORES)]
    res = run_bass_kernel_spmd(nc, in_maps, list(range(N_CORES)),
                               trace=_trace)
    total = sum(float(res.results[k]["loss"][0, 0]) for k in range(N_CORES))
    total += BATCH * N * (-LOG_LO)
    if _trace:
        _CACHED["last_exec_time_ns"] = res.exec_time_ns
    return np.array(total, dtype=np.float32)

